# revision 14
# baseline (speedup 1.0000x reference)
"""Trainium2 Bass kernel for block-causal masked multi-head self-attention.

Module: y = proj(softmax(mask(QK^T/sqrt(D))) V) for B=4, T=2048, C=512, H=8,
with a frame-block-causal mask (frame = t//4) and a per-key validity mask.

Sharding: 8 cores = 4 batches x 2 head-groups (4 heads each). Each core
computes QKV projections for its heads, flash-style attention, and a partial
output projection over its 256 channels; the host sums the two partial
projections per batch and adds the projection bias.

Device-side structure (all matmuls bf16 -> fp32 PSUM):
  - Scores are computed transposed (keys on partitions, queries on free dim).
  - The frame-causal mask inside a diagonal 128-block is folded into the
    QK^T matmul via 32 extra contraction rows: one-hot(frame(k)) on the K
    side against -640*[i > frame(q)] on the Q side.
  - Per-key validity masking zeroes rows of V' = [V | 1]; the appended
    ones-column makes the PV matmul produce softmax denominators for free
    (no max subtraction: scores are O(1)).
  - PV uses exp(scores) [keys, q] tiles as the *stationary* operand and V'
    [keys, 65] as the moving operand, so each PV matmul costs 65 moving
    columns instead of 512. The result lands as Y [q, 65] per query tile.
  - Normalization is one broadcast tensor_tensor multiply per (head, qc)
    (denominator reciprocals broadcast along a 0-stride axis).
  - Y [q, ch] tiles are transposed back to [ch, q] on the PE (for the
    output projection's contraction over channels) via is_transpose matmuls.
  - Phase A (projections) is interleaved with phase B (attention) per
    512-query chunk so the Activation engine starts exp'ing early.
"""

import math

import numpy as np

B, T, C = 4, 2048, 512
H, NOBJ, D = 8, 4, 64
NCORES = 8
HPC = 4  # heads per core
NKB = 16  # key blocks of 128
QCN = 4  # query chunks of 512
GRP = 2  # full key-blocks per PSUM score group ([128, 1024] = 2 banks)

_CACHE = {}


def _apply_tile_patch(tile_mod, mybir):
    """walrus in this container rejects >1 semaphore wait per instruction;
    spread the TileContext tail-drain waits over sync NOPs (the rest of the
    module is handled by _split_multi_waits after lowering)."""
    import bass_rust

    if getattr(tile_mod.TileContext, "_drain_patched", False):
        return

    def _drain_and_barrier(self, tick_clock, wait_clock):
        nc = self.nc
        drain_inst = nc.sync.drain()
        wait_clock.add_sem_waits(
            drain_inst.ins, bass_rust.ScopedClock({None: tick_clock.global_clock})
        )
        waits = list(drain_inst.ins.sync_info.on_wait or [])
        if len(waits) > 1:
            drain_inst.ins.sync_info.on_wait = []
            for w in waits:
                nop = nc.sync.nop(nofuse=True)
                nop.ins.sync_info = mybir.SyncInfo(on_wait=[w], on_update=[])
            nc.sync.drain()
        nc.all_engine_barrier()
        assert self.sems is not None
        popped = nc._tile_sem_poison_stack.pop()
        assert popped is self._sem_poison
        nc.clear_and_free_semaphores(list(self.sems.allocated().values()))
        nc.all_engine_barrier()

    tile_mod.TileContext._drain_and_barrier = _drain_and_barrier
    tile_mod.TileContext._drain_patched = True


def _split_multi_waits(nc, mybir):
    """Post-pass: for every instruction carrying more than one semaphore
    wait, hoist the extra waits onto same-engine NOPs inserted immediately
    before it (engines execute serially, so blocking at the NOP is
    equivalent)."""
    nonce = 0
    for fn in nc.m.functions:
        for blk in fn.blocks:
            insts = list(blk.instructions)
            out = []
            changed = False
            for ins in insts:
                si = ins.sync_info
                waits = list(si.on_wait) if si and si.on_wait else []
                if len(waits) > 1:
                    changed = True
                    for w in waits[:-1]:
                        nop = mybir.InstNoOp(
                            name=f"I-waitsplit-{nonce}", ins=[], outs=[]
                        )
                        nonce += 1
                        nop.engine = ins.engine
                        nop.sync_info = mybir.SyncInfo(on_wait=[w], on_update=[])
                        nc.register_instruction(nop, overwrite=True)
                        out.append(nop)
                    ins.sync_info.on_wait = waits[-1:]
                out.append(ins)
            if changed:
                blk.instructions = out


def _build_program():
    import concourse.bass as bass
    import concourse.mybir as mybir
    import concourse.tile as tile
    from concourse import masks

    _apply_tile_patch(tile, mybir)

    f32 = mybir.dt.float32
    bf16 = mybir.dt.bfloat16
    EXP = mybir.ActivationFunctionType.Exp

    nc = bass.Bass(trn_type="TRN2")

    xt = nc.dram_tensor("xt", [C, T], bf16, kind="ExternalInput")
    wqk = nc.dram_tensor("wqk", [C, 512], bf16, kind="ExternalInput")
    wv = nc.dram_tensor("wv", [C, 260], bf16, kind="ExternalInput")
    kf32 = nc.dram_tensor("kf32", [4 + NKB, 128], f32, kind="ExternalInput")
    wp = nc.dram_tensor("wp", [256, 512], bf16, kind="ExternalInput")
    aq = nc.dram_tensor("aq", [32, T], bf16, kind="ExternalInput")
    ak = nc.dram_tensor("ak", [32, T], bf16, kind="ExternalInput")
    konst = nc.dram_tensor("konst", [1, 1024], bf16, kind="ExternalInput")
    out = nc.dram_tensor("out", [T, C], f32, kind="ExternalOutput")

    def mm(o, lhsT, rhs, start, stop):
        nc.tensor.matmul(o, lhsT, rhs, start=start, stop=stop)

    with nc.allow_low_precision(
        reason="bf16 matmul inputs; PSUM accumulation stays fp32"
    ), tile.TileContext(nc) as tc:
        with tc.tile_pool(name="const", bufs=1) as cp:
            wqk_s = cp.tile([128, 4 * 512], bf16)
            wv_s = cp.tile([128, 4 * 260], bf16)
            wp_s = cp.tile([128, 2 * 512], bf16)
            kf_s = cp.tile([128, 4 + NKB], f32)
            kb_s = cp.tile([1, 1024], bf16)
            ident = cp.tile([128, 128], bf16)
            qtd = [cp.tile([128, T], bf16, tag=f"qtd{h}", name=f"qtd{h}") for h in range(HPC)]
            ktd = [cp.tile([128, T], bf16, tag=f"ktd{h}", name=f"ktd{h}") for h in range(HPC)]
            v4 = cp.tile([128, NKB * 260], bf16)
            xt_s = cp.tile([128, 4 * T], bf16)
            bqk_s = kf_s[:, 0:4]
            vm_s = kf_s[:, 4 : 4 + NKB]
            ones_s = kb_s[0:1, 0:512]
            bv_s = kb_s[0:1, 512:772]

            # critical-path-first DMA order: QK weights, x, head-0 one-hot
            # tiles, then the rest.
            for kc in range(4):
                nc.scalar.dma_start(wqk_s[:, kc * 512 : (kc + 1) * 512], wqk[kc * 128 : (kc + 1) * 128, :])
            def load_x(qc):
                for kc in range(4):
                    eng = nc.sync if kc % 2 == 0 else nc.scalar
                    eng.dma_start(
                        xt_s[:, kc * T + qc * 512 : kc * T + (qc + 1) * 512],
                        xt[kc * 128 : (kc + 1) * 128, qc * 512 : (qc + 1) * 512],
                    )

            load_x(0)
            nc.gpsimd.dma_start(kf_s[:], kf32[:].rearrange("n p -> p n"))
            nc.sync.dma_start(qtd[0][64:96, :], aq[:])
            nc.sync.dma_start(ktd[0][64:96, :], ak[:])
            nc.sync.dma_start(kb_s[:], konst[:])
            load_x(1)
            for kc in range(4):
                nc.scalar.dma_start(wv_s[:, kc * 260 : (kc + 1) * 260], wv[kc * 128 : (kc + 1) * 128, :])
            masks.make_identity(nc, ident[:])
            load_x(2)
            load_x(3)
            for h in range(1, HPC):
                nc.scalar.dma_start(qtd[h][64:96, :], aq[:])
                nc.scalar.dma_start(ktd[h][64:96, :], ak[:])
            for rc in range(2):
                nc.sync.dma_start(wp_s[:, rc * 512 : (rc + 1) * 512], wp[rc * 128 : (rc + 1) * 128, :])

            with tc.tile_pool(name="sbw", bufs=1) as sbw, tc.tile_pool(
                name="psw", bufs=1, space="PSUM"
            ) as psw:

                def aqk_unit(qc, bcol, rc):
                    """Q or K projection for one 128-channel half (2 heads)."""
                    dst = qtd if bcol == 0 else ktd
                    ps = psw.tile([128, 512], f32, tag="pb", bufs=2, name=f"pj{qc}{bcol}{rc}")
                    for kc in range(4):
                        mm(
                            ps[:],
                            wqk_s[:, kc * 512 + bcol * 256 + rc * 128 : kc * 512 + bcol * 256 + rc * 128 + 128],
                            xt_s[:, kc * T + qc * 512 : kc * T + (qc + 1) * 512],
                            kc == 0,
                            kc == 3,
                        )
                    for hh in range(2):
                        h = 2 * rc + hh
                        nc.vector.tensor_scalar_add(
                            dst[h][0:64, qc * 512 : (qc + 1) * 512],
                            ps[hh * 64 : (hh + 1) * 64, :],
                            bqk_s[hh * 64 : (hh + 1) * 64, 2 * bcol + rc : 2 * bcol + rc + 1],
                        )

                def v_unit(qc, j):
                    """V projection + validity mask for key block 4qc+j."""
                    kb = 4 * qc + j
                    pv = psw.tile([128, 260], f32, tag="pjv", bufs=1, name=f"pv{kb}")
                    mm(pv[:], ones_s[0:1, 0:128], bv_s[:], True, False)
                    for kc in range(4):
                        mm(
                            pv[:],
                            xt_s[:, kc * T + kb * 128 : kc * T + kb * 128 + 128],
                            wv_s[:, kc * 260 : (kc + 1) * 260],
                            False,
                            kc == 3,
                        )
                    nc.vector.tensor_scalar_mul(
                        v4[:, kb * 260 : (kb + 1) * 260], pv[:], vm_s[:, kb : kb + 1]
                    )

                def score_units(h, qc):
                    """Unit closures for QK^T + exp of head h, chunk qc."""
                    info = {"p_full": [], "p_diag": [], "d_off": {}, "f_off": {}}
                    units = []
                    qsl = slice(qc * 512, (qc + 1) * 512)
                    full_kbs = list(range(4 * qc))

                    def full_unit(kbs, g0):
                        at = psw.tile([128, GRP * 512], f32, tag="att", bufs=2, name=f"at{qc}{h}{g0}")
                        for i, kb in enumerate(kbs):
                            mm(
                                at[:, i * 512 : (i + 1) * 512],
                                ktd[h][0:64, kb * 128 : (kb + 1) * 128],
                                qtd[h][0:64, qsl],
                                True,
                                True,
                            )
                        p_t = sbw.tile([128, GRP * 512], bf16, tag="p", bufs=16, name=f"p{qc}{h}{g0}")
                        nc.scalar.activation(p_t[:, 0 : len(kbs) * 512], at[:, 0 : len(kbs) * 512], EXP)
                        info["p_full"].append(p_t)

                    for g0 in range(0, len(full_kbs), GRP):
                        kbs = full_kbs[g0 : g0 + GRP]
                        units.append(lambda kbs=kbs, g0=g0: full_unit(kbs, g0))

                    def diag_unit(js):
                        at = psw.tile([128, GRP * 512], f32, tag="att", bufs=2, name=f"atd{qc}{h}{js[0]}")
                        off = 0
                        for j in js:
                            kb = 4 * qc + j
                            mm(
                                at[:, off : off + 128],
                                ktd[h][0:96, kb * 128 : (kb + 1) * 128],
                                qtd[h][0:96, qc * 512 + j * 128 : qc * 512 + j * 128 + 128],
                                True,
                                True,
                            )
                            info["d_off"][j] = off
                            off += 128
                            fw = 512 - (j + 1) * 128
                            if fw > 0:
                                mm(
                                    at[:, off : off + fw],
                                    ktd[h][0:64, kb * 128 : (kb + 1) * 128],
                                    qtd[h][0:64, qc * 512 + (j + 1) * 128 : qc * 512 + (j + 1) * 128 + fw],
                                    True,
                                    True,
                                )
                                info["f_off"][j] = off
                                off += fw
                        p_t = sbw.tile([128, GRP * 512], bf16, tag="p", bufs=16, name=f"pd{qc}{h}{js[0]}")
                        nc.scalar.activation(p_t[:, 0:off], at[:, 0:off], EXP)
                        info["p_diag"].append(p_t)

                    for js in ((0, 1), (2, 3)):
                        units.append(lambda js=js: diag_unit(js))
                    return units, info

                def pv_units(h, qc, info, yh2, per_qt_norm=False):
                    """Unit closures: per-query-tile PV chains + normalize."""
                    units = []
                    full_kbs = list(range(4 * qc))

                    def chain(qt):
                        if qt == 0:
                            info["yt"] = psw.tile([128, 260], f32, tag="yt", bufs=1, name=f"yt{qc}{h}")
                        yt = info["yt"]
                        first = True
                        for kb in full_kbs:
                            mm(
                                yt[:, qt * 65 : qt * 65 + 65],
                                info["p_full"][kb // GRP][:, (kb % GRP) * 512 + qt * 128 : (kb % GRP) * 512 + qt * 128 + 128],
                                v4[:, kb * 260 + h * 65 : kb * 260 + h * 65 + 65],
                                first,
                                False,
                            )
                            first = False
                        for j in range(qt):
                            kb = 4 * qc + j
                            src = info["f_off"][j] + (qt - j - 1) * 128
                            mm(
                                yt[:, qt * 65 : qt * 65 + 65],
                                info["p_diag"][j // 2][:, src : src + 128],
                                v4[:, kb * 260 + h * 65 : kb * 260 + h * 65 + 65],
                                first,
                                False,
                            )
                            first = False
                        kb = 4 * qc + qt
                        mm(
                            yt[:, qt * 65 : qt * 65 + 65],
                            info["p_diag"][qt // 2][:, info["d_off"][qt] : info["d_off"][qt] + 128],
                            v4[:, kb * 260 + h * 65 : kb * 260 + h * 65 + 65],
                            first,
                            True,
                        )

                    for qt in range(4):
                        units.append(lambda qt=qt: chain(qt))

                    def norm(qts):
                        yt = info["yt"]
                        nq = len(qts)
                        q0 = qts[0]
                        pair, sl = h // 2, h % 2
                        rcp = sbw.tile([128, 4], f32, tag="rcp", bufs=2, name=f"rcp{qc}{h}{q0}")
                        if nq == 1:
                            nc.vector.reciprocal(rcp[:, 0:1], yt[:, q0 * 65 + 64 : q0 * 65 + 65])
                            nc.vector.tensor_scalar_mul(
                                yh2[pair][:, q0 * 128 + sl * 64 : q0 * 128 + sl * 64 + 64],
                                yt[:, q0 * 65 : q0 * 65 + 64],
                                rcp[:, 0:1],
                            )
                            return
                        yt3 = yt[:, q0 * 65 : (q0 + nq) * 65].rearrange("p (q x) -> p q x", x=65)
                        nc.vector.reciprocal(
                            rcp[:, 0:nq].rearrange("p (q x) -> p q x", x=1), yt3[:, :, 64:65]
                        )
                        dst = yh2[pair][:, q0 * 128 : (q0 + nq) * 128].rearrange(
                            "p (q x) -> p q x", x=128
                        )[:, :, sl * 64 : sl * 64 + 64]
                        nc.vector.tensor_mul(
                            dst, yt3[:, :, 0:64], rcp[:, 0:nq].broadcast_to([128, nq, 64])
                        )

                    if per_qt_norm:
                        for qt in range(4):
                            units.append(lambda qt=qt: norm([qt]))
                    else:
                        units.append(lambda: norm([0, 1, 2, 3]))
                    return units

                def issue_proj(qc, yh2, only_qt=None):
                    """Transpose Y to [ch, q] and project, per query tile."""
                    for qt in ([only_qt] if only_qt is not None else range(4)):
                        pst = psw.tile([128, 512], bf16, tag="att", bufs=2, name=f"pst{qc}{qt}")
                        for pair in range(2):
                            nc.tensor.matmul(
                                pst[:, pair * 128 : (pair + 1) * 128],
                                yh2[pair][:, qt * 128 : (qt + 1) * 128],
                                ident[:],
                                is_transpose=True,
                                start=True,
                                stop=True,
                            )
                        yhT = sbw.tile([128, 256], bf16, tag="yhT", bufs=3, name=f"yhT{qc}{qt}")
                        nc.vector.tensor_copy(yhT[:], pst[:, 0:256])
                        po = psw.tile([128, 512], f32, tag="pb", bufs=2, name=f"po{qc}{qt}")
                        for pair in range(2):
                            mm(
                                po[:],
                                yhT[:, pair * 128 : (pair + 1) * 128],
                                wp_s[:, pair * 512 : (pair + 1) * 512],
                                pair == 0,
                                pair == 1,
                            )
                        os_t = sbw.tile([128, 512], f32, tag="os", bufs=3, name=f"os{qc}{qt}")
                        nc.vector.tensor_copy(os_t[:], po[:])
                        nc.sync.dma_start(
                            out[(qc * 4 + qt) * 128 : (qc * 4 + qt + 1) * 128, :], os_t[:]
                        )

                def run_interleaved(s_units, p_units):
                    """S0 S1 P0 S2 P1 ... ; leftovers appended in order."""
                    si = pi = 0
                    lead = min(2, len(s_units))
                    while si < lead:
                        s_units[si]()
                        si += 1
                    while si < len(s_units) or pi < len(p_units):
                        if pi < len(p_units):
                            p_units[pi]()
                            pi += 1
                        if si < len(s_units):
                            s_units[si]()
                            si += 1

                # Deadline-tagged backlog of projection/filler work, spread
                # across head-blocks so PE always has sem-independent work
                # while the Activation engine drains the exp queue.
                backlog = []  # (deadline_block_index, unit)

                def push(dl, unit):
                    backlog.append((dl, unit))

                def run_due(bi):
                    rest = []
                    for dl, u in backlog:
                        if dl <= bi:
                            u()
                        else:
                            rest.append((dl, u))
                    backlog[:] = rest

                def take(n):
                    out = []
                    for _ in range(min(n, len(backlog))):
                        out.append(backlog.pop(0)[1])
                    return out

                def bi_of(qc, h):
                    return qc * 4 + h

                pend = None  # (h, qc, info, yh2)
                pend_proj = None  # (qc, yh2)
                aqk_unit(0, 0, 0)
                aqk_unit(0, 1, 0)
                push(bi_of(0, 2), lambda: aqk_unit(0, 0, 1))
                push(bi_of(0, 2), lambda: aqk_unit(0, 1, 1))
                for j in range(4):
                    push(bi_of(0, 1), lambda j=j: v_unit(0, j))
                for qc in range(QCN):
                    if qc + 1 < QCN:
                        nqc = qc + 1
                        push(bi_of(nqc, 0) - 1, lambda nqc=nqc: aqk_unit(nqc, 0, 0))
                        push(bi_of(nqc, 0) - 1, lambda nqc=nqc: aqk_unit(nqc, 1, 0))
                        push(bi_of(nqc, 1), lambda nqc=nqc: aqk_unit(nqc, 0, 1))
                        push(bi_of(nqc, 1), lambda nqc=nqc: aqk_unit(nqc, 1, 1))
                        for j in range(4):
                            push(bi_of(nqc, 1), lambda nqc=nqc, j=j: v_unit(nqc, j))
                    if pend_proj is not None:
                        push(bi_of(qc, 2), lambda pp=pend_proj: issue_proj(*pp))
                    yh2 = [
                        sbw.tile([128, 512], bf16, tag=f"yh{p}", bufs=2, name=f"yh{p}_{qc}")
                        for p in range(2)
                    ]
                    for h in range(HPC):
                        run_due(bi_of(qc, h))
                        s_units, info = score_units(h, qc)
                        fillers = list(pv_units(*pend)) if pend is not None else []
                        fillers += take(2)
                        run_interleaved(s_units, fillers)
                        pend = (h, qc, info, yh2)
                    pend_proj = (qc, yh2)
                run_due(10**9)
                # flush: last head's PV + norm + projection, pipelined per qt
                h, qc, info, yh2 = pend
                pv_flush = pv_units(h, qc, info, yh2, per_qt_norm=True)
                for qt in range(4):
                    pv_flush[qt]()  # chain
                    pv_flush[4 + qt]()  # per-qt norm
                    issue_proj(qc, yh2, only_qt=qt)
    _split_multi_waits(nc, mybir)
    return nc


def _host_inputs(x, mask, Wq, bq, Wk, bk, Wv, bv, Wp, bp):
    """Build the per-core input maps."""
    import ml_dtypes

    bf16 = ml_dtypes.bfloat16
    scale = 1.0 / math.sqrt(D)
    # one-hot / penalty patterns for the in-matmul diagonal causal mask
    u = np.arange(T) % 128
    fr = u // NOBJ  # frame within 128-tile, 0..31
    i_idx = np.arange(32)[:, None]
    ak_host = (fr[None, :] == i_idx).astype(bf16)
    aq_host = np.where(i_idx > fr[None, :], np.float32(-640.0), np.float32(0.0)).astype(bf16)
    konst_host = np.ones((1, 512), bf16)

    in_maps = []
    for c in range(NCORES):
        b, g = divmod(c, 2)
        ch = slice(g * 256, (g + 1) * 256)
        wq_h = np.ascontiguousarray((Wq[ch, :] * scale).T)  # [512, 256]
        wk_h = np.ascontiguousarray(Wk[ch, :].T)
        wqk_h = np.concatenate([wq_h, wk_h], axis=1)  # [512, 512]
        wv_flat = Wv[ch, :].T  # [512, 256]
        wv_h = np.zeros((C, 260), np.float32)
        bv_h = np.zeros(260, np.float32)
        bvc = bv[ch]
        for h in range(HPC):
            wv_h[:, h * 65 : h * 65 + 64] = wv_flat[:, h * 64 : (h + 1) * 64]
            bv_h[h * 65 : h * 65 + 64] = bvc[h * 64 : (h + 1) * 64]
            bv_h[h * 65 + 64] = 1.0
        bq_h = bq[ch] * scale
        bk_h = bk[ch]
        kf32_h = np.concatenate(
            [
                np.stack([bq_h[:128], bq_h[128:], bk_h[:128], bk_h[128:]]),
                mask[b].astype(np.float32).reshape(NKB, 128),
            ]
        )
        konst_h = np.zeros((1, 1024), np.float32)
        konst_h[0, 0:512] = 1.0
        konst_h[0, 512:772] = bv_h
        in_maps.append(
            {
                "xt": np.ascontiguousarray(x[b].T).astype(bf16),
                "wqk": wqk_h.astype(bf16),
                "wv": wv_h.astype(bf16),
                "kf32": kf32_h.astype(np.float32),
                "wp": np.ascontiguousarray(Wp[:, ch].T).astype(bf16),
                "aq": aq_host,
                "ak": ak_host,
                "konst": konst_h.astype(bf16),
            }
        )
    return in_maps


def kernel(x, mask, Wq, bq, Wk, bk, Wv, bv, Wp, bp):
    from concourse.bass_utils import run_bass_kernel_spmd

    if "nc" not in _CACHE:
        _CACHE["nc"] = _build_program()
    nc = _CACHE["nc"]

    in_maps = _host_inputs(
        np.asarray(x), np.asarray(mask),
        np.asarray(Wq), np.asarray(bq), np.asarray(Wk), np.asarray(bk),
        np.asarray(Wv), np.asarray(bv), np.asarray(Wp), np.asarray(bp),
    )
    res = run_bass_kernel_spmd(nc, in_maps, core_ids=list(range(NCORES)))
    outs = [res.results[c]["out"] for c in range(NCORES)]
    y = np.empty((B, T, C), np.float32)
    for b in range(B):
        y[b] = outs[2 * b] + outs[2 * b + 1] + np.asarray(bp)[None, :]
    return y
